# revision 5
# baseline (speedup 1.0000x reference)
"""ContrastMemory kernel for 8 Trainium2 NeuronCores.

Strategy (scatter_memory, memory-bound regime):
  out_v2[b,k] = exp(dot(memory_v1[idx[b,k]], v2[b]) / T)  (and symmetrically
  for out_v1).  Instead of gathering 2*B*(K+1) rows (537 MB of random-access
  HBM traffic), compute the dense score matrices S1 = memory_v1 @ v2.T and
  S2 = memory_v2 @ v1.T on-device (row-sharded across the 8 cores, 12500
  rows each), apply exp(x/T) on the Scalar engine, and stream the dense
  exp-matrices back.  The host then assembles the outputs by indexing the
  dense matrices (pure data movement), computes the Z constants from the
  gathered values, and splices the device-computed momentum-updated rows
  into the memory banks (y = arange(B), so the scatter targets are rows
  0..127, which are disjoint per sample).

Per-core device work: read 12.8 MB (two transposed bank shards), two
float32r matmul chains (12500 columns each), exp on ACT, write 12.8 MB.
"""

import os
import sys

sys.path.insert(0, "/opt/trn_rl_repo")

import numpy as np

N = 100000
D = 128
B = 128
K1 = 4097  # K+1
T = 0.07
N_CORES = 8
ROWS = N // N_CORES  # 12500
CHUNK = 500
N_CHUNKS = ROWS // CHUNK  # 25

_COMPILED = None


def _build():
    from concourse import bacc, tile, mybir

    nc = bacc.Bacc("TRN2", target_bir_lowering=False, debug=False, num_devices=1)
    dt = mybir.dt

    # per-core inputs (float32r = fp32 bits, full-rate PE path)
    memT1 = nc.dram_tensor("memT1", [D, ROWS], dt.float32r, kind="ExternalInput")
    memT2 = nc.dram_tensor("memT2", [D, ROWS], dt.float32r, kind="ExternalInput")
    vT1 = nc.dram_tensor("vT1", [D, B], dt.float32r, kind="ExternalInput")
    vT2 = nc.dram_tensor("vT2", [D, B], dt.float32r, kind="ExternalInput")
    mh1 = nc.dram_tensor("mh1", [B, D], dt.float32, kind="ExternalInput")  # memory_v1[:128]
    mh2 = nc.dram_tensor("mh2", [B, D], dt.float32, kind="ExternalInput")
    v1in = nc.dram_tensor("v1in", [B, D], dt.float32, kind="ExternalInput")
    v2in = nc.dram_tensor("v2in", [B, D], dt.float32, kind="ExternalInput")

    # per-core outputs
    E1 = nc.dram_tensor("E1", [B, ROWS], dt.float32, kind="ExternalOutput")
    E2 = nc.dram_tensor("E2", [B, ROWS], dt.float32, kind="ExternalOutput")
    upd1 = nc.dram_tensor("upd1", [B, D], dt.float32, kind="ExternalOutput")
    upd2 = nc.dram_tensor("upd2", [B, D], dt.float32, kind="ExternalOutput")

    f32 = dt.float32
    f32r = dt.float32r
    Exp = mybir.ActivationFunctionType.Exp

    with tile.TileContext(nc) as tc:
        with (
            tc.tile_pool(name="banks", bufs=1) as banks,
            tc.tile_pool(name="vecs", bufs=1) as vecs,
            tc.tile_pool(name="psum", bufs=4, space="PSUM") as psum,
            tc.tile_pool(name="eout", bufs=4) as eout,
            tc.tile_pool(name="small", bufs=1) as small,
        ):
            t_vT1 = vecs.tile([D, B], f32r, tag="v1")
            t_vT2 = vecs.tile([D, B], f32r, tag="v2")
            nc.sync.dma_start(out=t_vT1[:], in_=vT1.ap())
            nc.sync.dma_start(out=t_vT2[:], in_=vT2.ap())

            # load the bank shards in LCHUNK-column pieces, interleaved
            # across banks, so the matmul pipeline starts after ~1 MB of
            # DMA instead of waiting for the full 12.8 MB
            LCHUNK = 2500
            NL = ROWS // LCHUNK  # 5
            SUB = LCHUNK // CHUNK  # 5
            mem_tiles = {}
            for li in range(NL):
                for bank in range(2):
                    src = memT1 if bank == 0 else memT2
                    t = banks.tile([D, LCHUNK], f32r, tag=f"m{bank}_{li}")
                    nc.sync.dma_start(
                        out=t[:], in_=src.ap()[:, li * LCHUNK : (li + 1) * LCHUNK]
                    )
                    mem_tiles[(bank, li)] = t

            for li in range(NL):
                for bank in range(2):
                    t_mem = mem_tiles[(bank, li)]
                    t_v = t_vT2 if bank == 0 else t_vT1
                    e_dram = E1 if bank == 0 else E2
                    for sub in range(SUB):
                        ci = li * SUB + sub
                        ps = psum.tile([B, CHUNK], f32, tag="ps")
                        nc.tensor.matmul(
                            ps[:],
                            t_v[:],
                            t_mem[:, sub * CHUNK : (sub + 1) * CHUNK],
                            start=True,
                            stop=True,
                        )
                        te = eout.tile([B, CHUNK], f32, tag="e")
                        nc.scalar.activation(te[:], ps[:], Exp, scale=1.0 / T)
                        nc.sync.dma_start(
                            out=e_dram.ap()[:, ci * CHUNK : (ci + 1) * CHUNK],
                            in_=te[:],
                        )

            # momentum update rows (y = arange(B) -> bank rows 0..127):
            # upd = (m + v) / ||m + v||  (the 0.5 momentum factor cancels)
            for bank in range(2):
                t_mh = small.tile([B, D], f32, tag=f"mh{bank}")
                t_vv = small.tile([B, D], f32, tag=f"vv{bank}")
                nc.sync.dma_start(out=t_mh[:], in_=(mh1 if bank == 0 else mh2).ap())
                nc.sync.dma_start(out=t_vv[:], in_=(v1in if bank == 0 else v2in).ap())
                t_sum = small.tile([B, D], f32, tag=f"sum{bank}")
                t_sq = small.tile([B, D], f32, tag=f"sq{bank}")
                t_ss = small.tile([B, 1], f32, tag=f"ss{bank}")
                nc.vector.tensor_tensor(t_sum[:], t_mh[:], t_vv[:], mybir.AluOpType.add)
                nc.vector.tensor_tensor(t_sq[:], t_sum[:], t_sum[:], mybir.AluOpType.mult)
                nc.vector.tensor_reduce(
                    t_ss[:], t_sq[:], mybir.AxisListType.XYZW, mybir.AluOpType.add
                )
                t_norm = small.tile([B, 1], f32, tag=f"n{bank}")
                t_rn = small.tile([B, 1], f32, tag=f"rn{bank}")
                nc.scalar.sqrt(t_norm[:], t_ss[:])
                nc.vector.reciprocal(t_rn[:], t_norm[:])
                t_out = small.tile([B, D], f32, tag=f"o{bank}")
                nc.scalar.activation(
                    t_out[:], t_sum[:], mybir.ActivationFunctionType.Copy,
                    scale=t_rn[:],
                )
                nc.sync.dma_start(
                    out=(upd1 if bank == 0 else upd2).ap(), in_=t_out[:]
                )

    nc.compile()
    return nc


def _get_compiled():
    global _COMPILED
    if _COMPILED is None:
        _COMPILED = _build()
    return _COMPILED


def kernel(v1, v2, idx, y, memory_v1, memory_v2):
    from concourse.bass_utils import run_bass_kernel_spmd

    v1 = np.asarray(v1, dtype=np.float32)
    v2 = np.asarray(v2, dtype=np.float32)
    memory_v1 = np.asarray(memory_v1, dtype=np.float32)
    memory_v2 = np.asarray(memory_v2, dtype=np.float32)
    idx_np = np.asarray(idx)
    y_np = np.asarray(y)

    nc = _get_compiled()

    vT1 = np.ascontiguousarray(v1.T)
    vT2 = np.ascontiguousarray(v2.T)
    mh1 = np.ascontiguousarray(memory_v1[:B])
    mh2 = np.ascontiguousarray(memory_v2[:B])

    in_maps = []
    for c in range(N_CORES):
        r0, r1 = c * ROWS, (c + 1) * ROWS
        in_maps.append({
            "memT1": np.ascontiguousarray(memory_v1[r0:r1].T),
            "memT2": np.ascontiguousarray(memory_v2[r0:r1].T),
            "vT1": vT1, "vT2": vT2,
            "mh1": mh1, "mh2": mh2,
            "v1in": v1, "v2in": v2,
        })

    trace = bool(int(os.environ.get("TRNK_TRACE", "0")))
    if trace:
        try:
            sys.path.insert(0, "/root/.axon_site")
            from trn_agent_boot.trn_boot import _ntff_profile_via_ctypes
            from antenv.axon_hooks import set_axon_ntff_profile_hook
            set_axon_ntff_profile_hook(
                _ntff_profile_via_ctypes("/opt/axon/libaxon_pjrt.so"))
        except Exception as e:
            print(f"trace hook unavailable: {e}")
            trace = False
    res = run_bass_kernel_spmd(nc, in_maps, list(range(N_CORES)), trace=trace)
    if trace:
        kernel.last_exec_ns = res.exec_time_ns

    # ---- host-side unshard/assembly ----
    E1 = np.concatenate([res.results[c]["E1"] for c in range(N_CORES)], axis=1)
    E2 = np.concatenate([res.results[c]["E2"] for c in range(N_CORES)], axis=1)

    out_v2 = np.take_along_axis(E1, idx_np.astype(np.int64), axis=1)
    out_v1 = np.take_along_axis(E2, idx_np.astype(np.int64), axis=1)
    Z_v1 = np.float32(out_v1.mean()) * np.float32(N)
    Z_v2 = np.float32(out_v2.mean()) * np.float32(N)
    out_v1 = (out_v1 / Z_v1).astype(np.float32)
    out_v2 = (out_v2 / Z_v2).astype(np.float32)

    new_memory_v1 = memory_v1.copy()
    new_memory_v2 = memory_v2.copy()
    # y = arange(B) per the problem spec; honor arbitrary y defensively.
    new_memory_v1[y_np] = res.results[0]["upd1"]
    new_memory_v2[y_np] = res.results[0]["upd2"]

    return out_v1, out_v2, new_memory_v1, new_memory_v2


kernel.last_exec_ns = None


# revision 6
# speedup vs baseline: 1.0758x; 1.0758x over previous
"""ContrastMemory kernel for 8 Trainium2 NeuronCores.

Strategy (scatter_memory, memory-bound regime):
  out_v2[b,k] = exp(dot(memory_v1[idx[b,k]], v2[b]) / T)  (and symmetrically
  for out_v1).  Instead of gathering 2*B*(K+1) rows (537 MB of random-access
  HBM traffic), compute the dense score matrices S1 = memory_v1 @ v2.T and
  S2 = memory_v2 @ v1.T on-device (row-sharded across the 8 cores, 12500
  rows each), apply exp(x/T) on the Scalar engine, and stream the dense
  exp-matrices back.  The host then assembles the outputs by indexing the
  dense matrices (pure data movement), computes the Z constants from the
  gathered values, and splices the device-computed momentum-updated rows
  into the memory banks (y = arange(B), so the scatter targets are rows
  0..127, which are disjoint per sample).

Per-core device work: read 12.8 MB (two transposed bank shards), two
float32r matmul chains (12500 columns each), exp on ACT, write 12.8 MB.
"""

import os
import sys

sys.path.insert(0, "/opt/trn_rl_repo")

import numpy as np

N = 100000
D = 128
B = 128
K1 = 4097  # K+1
T = 0.07
N_CORES = 8
ROWS = N // N_CORES  # 12500
CHUNK = 500
N_CHUNKS = ROWS // CHUNK  # 25

_COMPILED = None


def _build():
    from concourse import bacc, tile, mybir

    nc = bacc.Bacc("TRN2", target_bir_lowering=False, debug=False, num_devices=1)
    dt = mybir.dt

    # per-core inputs (float32r = fp32 bits, full-rate PE path)
    memT1 = nc.dram_tensor("memT1", [D, ROWS], dt.float32r, kind="ExternalInput")
    memT2 = nc.dram_tensor("memT2", [D, ROWS], dt.float32r, kind="ExternalInput")
    vT1 = nc.dram_tensor("vT1", [D, B], dt.float32r, kind="ExternalInput")
    vT2 = nc.dram_tensor("vT2", [D, B], dt.float32r, kind="ExternalInput")
    mh1 = nc.dram_tensor("mh1", [B, D], dt.float32, kind="ExternalInput")  # memory_v1[:128]
    mh2 = nc.dram_tensor("mh2", [B, D], dt.float32, kind="ExternalInput")
    v1in = nc.dram_tensor("v1in", [B, D], dt.float32, kind="ExternalInput")
    v2in = nc.dram_tensor("v2in", [B, D], dt.float32, kind="ExternalInput")

    # per-core outputs
    E1 = nc.dram_tensor("E1", [B, ROWS], dt.bfloat16, kind="ExternalOutput")
    E2 = nc.dram_tensor("E2", [B, ROWS], dt.bfloat16, kind="ExternalOutput")
    upd1 = nc.dram_tensor("upd1", [B, D], dt.float32, kind="ExternalOutput")
    upd2 = nc.dram_tensor("upd2", [B, D], dt.float32, kind="ExternalOutput")

    f32 = dt.float32
    f32r = dt.float32r
    Exp = mybir.ActivationFunctionType.Exp

    with tile.TileContext(nc) as tc:
        with (
            tc.tile_pool(name="banks", bufs=1) as banks,
            tc.tile_pool(name="vecs", bufs=1) as vecs,
            tc.tile_pool(name="psum", bufs=4, space="PSUM") as psum,
            tc.tile_pool(name="eout", bufs=4) as eout,
            tc.tile_pool(name="small", bufs=1) as small,
        ):
            t_vT1 = vecs.tile([D, B], f32r, tag="v1")
            t_vT2 = vecs.tile([D, B], f32r, tag="v2")
            nc.sync.dma_start(out=t_vT1[:], in_=vT1.ap())
            nc.sync.dma_start(out=t_vT2[:], in_=vT2.ap())

            # load the bank shards in LCHUNK-column pieces, interleaved
            # across banks, so the matmul pipeline starts after ~1 MB of
            # DMA instead of waiting for the full 12.8 MB
            LCHUNK = 2500
            NL = ROWS // LCHUNK  # 5
            SUB = LCHUNK // CHUNK  # 5
            mem_tiles = {}
            for li in range(NL):
                for bank in range(2):
                    src = memT1 if bank == 0 else memT2
                    t = banks.tile([D, LCHUNK], f32r, tag=f"m{bank}_{li}")
                    nc.sync.dma_start(
                        out=t[:], in_=src.ap()[:, li * LCHUNK : (li + 1) * LCHUNK]
                    )
                    mem_tiles[(bank, li)] = t

            for li in range(NL):
                for bank in range(2):
                    t_mem = mem_tiles[(bank, li)]
                    t_v = t_vT2 if bank == 0 else t_vT1
                    e_dram = E1 if bank == 0 else E2
                    for sub in range(SUB):
                        ci = li * SUB + sub
                        ps = psum.tile([B, CHUNK], f32, tag="ps")
                        nc.tensor.matmul(
                            ps[:],
                            t_v[:],
                            t_mem[:, sub * CHUNK : (sub + 1) * CHUNK],
                            start=True,
                            stop=True,
                        )
                        te = eout.tile([B, CHUNK], dt.bfloat16, tag="e")
                        nc.scalar.activation(te[:], ps[:], Exp, scale=1.0 / T)
                        nc.sync.dma_start(
                            out=e_dram.ap()[:, ci * CHUNK : (ci + 1) * CHUNK],
                            in_=te[:],
                        )

            # momentum update rows (y = arange(B) -> bank rows 0..127):
            # upd = (m + v) / ||m + v||  (the 0.5 momentum factor cancels)
            for bank in range(2):
                t_mh = small.tile([B, D], f32, tag=f"mh{bank}")
                t_vv = small.tile([B, D], f32, tag=f"vv{bank}")
                nc.sync.dma_start(out=t_mh[:], in_=(mh1 if bank == 0 else mh2).ap())
                nc.sync.dma_start(out=t_vv[:], in_=(v1in if bank == 0 else v2in).ap())
                t_sum = small.tile([B, D], f32, tag=f"sum{bank}")
                t_sq = small.tile([B, D], f32, tag=f"sq{bank}")
                t_ss = small.tile([B, 1], f32, tag=f"ss{bank}")
                nc.vector.tensor_tensor(t_sum[:], t_mh[:], t_vv[:], mybir.AluOpType.add)
                nc.vector.tensor_tensor(t_sq[:], t_sum[:], t_sum[:], mybir.AluOpType.mult)
                nc.vector.tensor_reduce(
                    t_ss[:], t_sq[:], mybir.AxisListType.XYZW, mybir.AluOpType.add
                )
                t_norm = small.tile([B, 1], f32, tag=f"n{bank}")
                t_rn = small.tile([B, 1], f32, tag=f"rn{bank}")
                nc.scalar.sqrt(t_norm[:], t_ss[:])
                nc.vector.reciprocal(t_rn[:], t_norm[:])
                t_out = small.tile([B, D], f32, tag=f"o{bank}")
                nc.scalar.activation(
                    t_out[:], t_sum[:], mybir.ActivationFunctionType.Copy,
                    scale=t_rn[:],
                )
                nc.sync.dma_start(
                    out=(upd1 if bank == 0 else upd2).ap(), in_=t_out[:]
                )

    nc.compile()
    return nc


def _get_compiled():
    global _COMPILED
    if _COMPILED is None:
        _COMPILED = _build()
    return _COMPILED


def kernel(v1, v2, idx, y, memory_v1, memory_v2):
    from concourse.bass_utils import run_bass_kernel_spmd

    v1 = np.asarray(v1, dtype=np.float32)
    v2 = np.asarray(v2, dtype=np.float32)
    memory_v1 = np.asarray(memory_v1, dtype=np.float32)
    memory_v2 = np.asarray(memory_v2, dtype=np.float32)
    idx_np = np.asarray(idx)
    y_np = np.asarray(y)

    nc = _get_compiled()

    vT1 = np.ascontiguousarray(v1.T)
    vT2 = np.ascontiguousarray(v2.T)
    mh1 = np.ascontiguousarray(memory_v1[:B])
    mh2 = np.ascontiguousarray(memory_v2[:B])

    in_maps = []
    for c in range(N_CORES):
        r0, r1 = c * ROWS, (c + 1) * ROWS
        in_maps.append({
            "memT1": np.ascontiguousarray(memory_v1[r0:r1].T),
            "memT2": np.ascontiguousarray(memory_v2[r0:r1].T),
            "vT1": vT1, "vT2": vT2,
            "mh1": mh1, "mh2": mh2,
            "v1in": v1, "v2in": v2,
        })

    trace = bool(int(os.environ.get("TRNK_TRACE", "0")))
    if trace:
        try:
            sys.path.insert(0, "/root/.axon_site")
            from trn_agent_boot.trn_boot import _ntff_profile_via_ctypes
            from antenv.axon_hooks import set_axon_ntff_profile_hook
            set_axon_ntff_profile_hook(
                _ntff_profile_via_ctypes("/opt/axon/libaxon_pjrt.so"))
        except Exception as e:
            print(f"trace hook unavailable: {e}")
            trace = False
    res = run_bass_kernel_spmd(nc, in_maps, list(range(N_CORES)), trace=trace)
    if trace:
        kernel.last_exec_ns = res.exec_time_ns

    # ---- host-side unshard/assembly ----
    E1 = np.concatenate(
        [res.results[c]["E1"].astype(np.float32) for c in range(N_CORES)], axis=1)
    E2 = np.concatenate(
        [res.results[c]["E2"].astype(np.float32) for c in range(N_CORES)], axis=1)

    out_v2 = np.take_along_axis(E1, idx_np.astype(np.int64), axis=1)
    out_v1 = np.take_along_axis(E2, idx_np.astype(np.int64), axis=1)
    Z_v1 = np.float32(out_v1.mean()) * np.float32(N)
    Z_v2 = np.float32(out_v2.mean()) * np.float32(N)
    out_v1 = (out_v1 / Z_v1).astype(np.float32)
    out_v2 = (out_v2 / Z_v2).astype(np.float32)

    new_memory_v1 = memory_v1.copy()
    new_memory_v2 = memory_v2.copy()
    # y = arange(B) per the problem spec; honor arbitrary y defensively.
    new_memory_v1[y_np] = res.results[0]["upd1"]
    new_memory_v2[y_np] = res.results[0]["upd2"]

    return out_v1, out_v2, new_memory_v1, new_memory_v2


kernel.last_exec_ns = None


# revision 8
# speedup vs baseline: 1.0958x; 1.0185x over previous
"""ContrastMemory kernel for 8 Trainium2 NeuronCores.

Strategy (scatter_memory, memory-bound regime):
  out_v2[b,k] = exp(dot(memory_v1[idx[b,k]], v2[b]) / T)  (and symmetrically
  for out_v1).  Instead of gathering 2*B*(K+1) rows (537 MB of random-access
  HBM traffic), compute the dense score matrices S1 = memory_v1 @ v2.T and
  S2 = memory_v2 @ v1.T on-device (row-sharded across the 8 cores, 12500
  rows each), apply exp(x/T) on the Scalar engine, and stream the dense
  exp-matrices back.  The host then assembles the outputs by indexing the
  dense matrices (pure data movement), computes the Z constants from the
  gathered values, and splices the device-computed momentum-updated rows
  into the memory banks (y = arange(B), so the scatter targets are rows
  0..127, which are disjoint per sample).

Per-core device work: read 12.8 MB (two transposed bank shards), two
float32r matmul chains (12500 columns each), exp on ACT, write 12.8 MB.
"""

import os
import sys

sys.path.insert(0, "/opt/trn_rl_repo")

import numpy as np

N = 100000
D = 128
B = 128
K1 = 4097  # K+1
T = 0.07
N_CORES = 8
ROWS = N // N_CORES  # 12500
CHUNK = 500
N_CHUNKS = ROWS // CHUNK  # 25

_COMPILED = None


def _build():
    from concourse import bacc, tile, mybir

    nc = bacc.Bacc("TRN2", target_bir_lowering=False, debug=False, num_devices=1)
    dt = mybir.dt

    # per-core inputs (float32r = fp32 bits, full-rate PE path)
    memT1 = nc.dram_tensor("memT1", [D, ROWS], dt.float32r, kind="ExternalInput")
    memT2 = nc.dram_tensor("memT2", [D, ROWS], dt.float32r, kind="ExternalInput")
    vT1 = nc.dram_tensor("vT1", [D, B], dt.float32r, kind="ExternalInput")
    vT2 = nc.dram_tensor("vT2", [D, B], dt.float32r, kind="ExternalInput")
    mh1 = nc.dram_tensor("mh1", [B, D], dt.float32, kind="ExternalInput")  # memory_v1[:128]
    mh2 = nc.dram_tensor("mh2", [B, D], dt.float32, kind="ExternalInput")
    v1in = nc.dram_tensor("v1in", [B, D], dt.float32, kind="ExternalInput")
    v2in = nc.dram_tensor("v2in", [B, D], dt.float32, kind="ExternalInput")

    # per-core outputs
    E1 = nc.dram_tensor("E1", [B, ROWS], dt.bfloat16, kind="ExternalOutput")
    E2 = nc.dram_tensor("E2", [B, ROWS], dt.bfloat16, kind="ExternalOutput")
    upd1 = nc.dram_tensor("upd1", [B, D], dt.float32, kind="ExternalOutput")
    upd2 = nc.dram_tensor("upd2", [B, D], dt.float32, kind="ExternalOutput")

    f32 = dt.float32
    f32r = dt.float32r
    Exp = mybir.ActivationFunctionType.Exp

    with tile.TileContext(nc) as tc:
        with (
            tc.tile_pool(name="banks", bufs=1) as banks,
            tc.tile_pool(name="vecs", bufs=1) as vecs,
            tc.tile_pool(name="psum", bufs=6, space="PSUM") as psum,
            tc.tile_pool(name="eout", bufs=10) as eout,
            tc.tile_pool(name="small", bufs=1) as small,
        ):
            t_vT1 = vecs.tile([D, B], f32r, tag="v1")
            t_vT2 = vecs.tile([D, B], f32r, tag="v2")
            nc.sync.dma_start(out=t_vT1[:], in_=vT1.ap())
            nc.sync.dma_start(out=t_vT2[:], in_=vT2.ap())

            # Stream the bank shards in LCHUNK-column pieces.  Each piece's
            # load is issued right before its matmuls so the sync-HWDGE
            # queue interleaves loads with compute instead of front-running
            # all 12.8 MB; E-writebacks go out on the scalar-HWDGE queue so
            # they never queue behind loads.
            LCHUNK = 2500
            NL = ROWS // LCHUNK  # 5
            SUB = LCHUNK // CHUNK  # 5
            for li in range(NL):
                for bank in range(2):
                    src = memT1 if bank == 0 else memT2
                    t_mem = banks.tile([D, LCHUNK], f32r, tag=f"m{bank}_{li}")
                    nc.sync.dma_start(
                        out=t_mem[:],
                        in_=src.ap()[:, li * LCHUNK : (li + 1) * LCHUNK],
                    )
                    t_v = t_vT2 if bank == 0 else t_vT1
                    e_dram = E1 if bank == 0 else E2
                    for sub in range(SUB):
                        ci = li * SUB + sub
                        ps = psum.tile([B, CHUNK], f32, tag="ps")
                        nc.tensor.matmul(
                            ps[:],
                            t_v[:],
                            t_mem[:, sub * CHUNK : (sub + 1) * CHUNK],
                            start=True,
                            stop=True,
                        )
                        te = eout.tile([B, CHUNK], dt.bfloat16, tag="e")
                        nc.scalar.activation(te[:], ps[:], Exp, scale=1.0 / T)
                        nc.scalar.dma_start(
                            out=e_dram.ap()[:, ci * CHUNK : (ci + 1) * CHUNK],
                            in_=te[:],
                        )

            # momentum update rows (y = arange(B) -> bank rows 0..127):
            # upd = (m + v) / ||m + v||  (the 0.5 momentum factor cancels)
            for bank in range(2):
                t_mh = small.tile([B, D], f32, tag=f"mh{bank}")
                t_vv = small.tile([B, D], f32, tag=f"vv{bank}")
                nc.sync.dma_start(out=t_mh[:], in_=(mh1 if bank == 0 else mh2).ap())
                nc.sync.dma_start(out=t_vv[:], in_=(v1in if bank == 0 else v2in).ap())
                t_sum = small.tile([B, D], f32, tag=f"sum{bank}")
                t_sq = small.tile([B, D], f32, tag=f"sq{bank}")
                t_ss = small.tile([B, 1], f32, tag=f"ss{bank}")
                nc.vector.tensor_tensor(t_sum[:], t_mh[:], t_vv[:], mybir.AluOpType.add)
                nc.vector.tensor_tensor(t_sq[:], t_sum[:], t_sum[:], mybir.AluOpType.mult)
                nc.vector.tensor_reduce(
                    t_ss[:], t_sq[:], mybir.AxisListType.XYZW, mybir.AluOpType.add
                )
                t_norm = small.tile([B, 1], f32, tag=f"n{bank}")
                t_rn = small.tile([B, 1], f32, tag=f"rn{bank}")
                nc.scalar.sqrt(t_norm[:], t_ss[:])
                nc.vector.reciprocal(t_rn[:], t_norm[:])
                t_out = small.tile([B, D], f32, tag=f"o{bank}")
                nc.scalar.activation(
                    t_out[:], t_sum[:], mybir.ActivationFunctionType.Copy,
                    scale=t_rn[:],
                )
                nc.sync.dma_start(
                    out=(upd1 if bank == 0 else upd2).ap(), in_=t_out[:]
                )

    nc.compile()
    return nc


def _get_compiled():
    global _COMPILED
    if _COMPILED is None:
        _COMPILED = _build()
    return _COMPILED


def kernel(v1, v2, idx, y, memory_v1, memory_v2):
    from concourse.bass_utils import run_bass_kernel_spmd

    v1 = np.asarray(v1, dtype=np.float32)
    v2 = np.asarray(v2, dtype=np.float32)
    memory_v1 = np.asarray(memory_v1, dtype=np.float32)
    memory_v2 = np.asarray(memory_v2, dtype=np.float32)
    idx_np = np.asarray(idx)
    y_np = np.asarray(y)

    nc = _get_compiled()

    vT1 = np.ascontiguousarray(v1.T)
    vT2 = np.ascontiguousarray(v2.T)
    mh1 = np.ascontiguousarray(memory_v1[:B])
    mh2 = np.ascontiguousarray(memory_v2[:B])

    in_maps = []
    for c in range(N_CORES):
        r0, r1 = c * ROWS, (c + 1) * ROWS
        in_maps.append({
            "memT1": np.ascontiguousarray(memory_v1[r0:r1].T),
            "memT2": np.ascontiguousarray(memory_v2[r0:r1].T),
            "vT1": vT1, "vT2": vT2,
            "mh1": mh1, "mh2": mh2,
            "v1in": v1, "v2in": v2,
        })

    trace = bool(int(os.environ.get("TRNK_TRACE", "0")))
    if trace:
        try:
            sys.path.insert(0, "/root/.axon_site")
            from trn_agent_boot.trn_boot import _ntff_profile_via_ctypes
            from antenv.axon_hooks import set_axon_ntff_profile_hook
            set_axon_ntff_profile_hook(
                _ntff_profile_via_ctypes("/opt/axon/libaxon_pjrt.so"))
        except Exception as e:
            print(f"trace hook unavailable: {e}")
            trace = False
    res = run_bass_kernel_spmd(nc, in_maps, list(range(N_CORES)), trace=trace)
    if trace:
        kernel.last_exec_ns = res.exec_time_ns

    # ---- host-side unshard/assembly ----
    E1 = np.concatenate(
        [res.results[c]["E1"].astype(np.float32) for c in range(N_CORES)], axis=1)
    E2 = np.concatenate(
        [res.results[c]["E2"].astype(np.float32) for c in range(N_CORES)], axis=1)

    out_v2 = np.take_along_axis(E1, idx_np.astype(np.int64), axis=1)
    out_v1 = np.take_along_axis(E2, idx_np.astype(np.int64), axis=1)
    Z_v1 = np.float32(out_v1.mean()) * np.float32(N)
    Z_v2 = np.float32(out_v2.mean()) * np.float32(N)
    out_v1 = (out_v1 / Z_v1).astype(np.float32)
    out_v2 = (out_v2 / Z_v2).astype(np.float32)

    new_memory_v1 = memory_v1.copy()
    new_memory_v2 = memory_v2.copy()
    # y = arange(B) per the problem spec; honor arbitrary y defensively.
    new_memory_v1[y_np] = res.results[0]["upd1"]
    new_memory_v2[y_np] = res.results[0]["upd2"]

    return out_v1, out_v2, new_memory_v1, new_memory_v2


kernel.last_exec_ns = None


# revision 10
# speedup vs baseline: 1.3523x; 1.2341x over previous
"""ContrastMemory kernel for 8 Trainium2 NeuronCores.

Strategy (scatter_memory, memory-bound regime):
  out_v2[b,k] = exp(dot(memory_v1[idx[b,k]], v2[b]) / T)  (and symmetrically
  for out_v1).  Instead of gathering 2*B*(K+1) rows (537 MB of random-access
  HBM traffic), compute the dense score matrices S1 = memory_v1 @ v2.T and
  S2 = memory_v2 @ v1.T on-device (row-sharded across the 8 cores, 12500
  rows each), apply exp(x/T) on the Scalar engine, and stream the dense
  exp-matrices back.  The host then assembles the outputs by indexing the
  dense matrices (pure data movement), computes the Z constants from the
  gathered values, and splices the device-computed momentum-updated rows
  into the memory banks (y = arange(B), so the scatter targets are rows
  0..127, which are disjoint per sample).

Per-core device work: read 12.8 MB (two transposed bank shards), two
float32r matmul chains (12500 columns each), exp on ACT, write 12.8 MB.
"""

import os
import sys

sys.path.insert(0, "/opt/trn_rl_repo")

import numpy as np

N = 100000
D = 128
B = 128
K1 = 4097  # K+1
T = 0.07
N_CORES = 8
ROWS = N // N_CORES  # 12500
CHUNK = 500
N_CHUNKS = ROWS // CHUNK  # 25

_COMPILED = None


def _build():
    from concourse import bacc, tile, mybir

    nc = bacc.Bacc("TRN2", target_bir_lowering=False, debug=False, num_devices=1)
    dt = mybir.dt

    # per-core inputs (float32r = fp32 bits, full-rate PE path)
    memT1 = nc.dram_tensor("memT1", [D, ROWS], dt.float32r, kind="ExternalInput")
    memT2 = nc.dram_tensor("memT2", [D, ROWS], dt.float32r, kind="ExternalInput")
    vT1 = nc.dram_tensor("vT1", [D, B], dt.float32r, kind="ExternalInput")
    vT2 = nc.dram_tensor("vT2", [D, B], dt.float32r, kind="ExternalInput")
    mh1 = nc.dram_tensor("mh1", [B, D], dt.float32, kind="ExternalInput")  # memory_v1[:128]
    mh2 = nc.dram_tensor("mh2", [B, D], dt.float32, kind="ExternalInput")
    v1in = nc.dram_tensor("v1in", [B, D], dt.float32, kind="ExternalInput")
    v2in = nc.dram_tensor("v2in", [B, D], dt.float32, kind="ExternalInput")

    # per-core outputs
    E1 = nc.dram_tensor("E1", [B, ROWS], dt.bfloat16, kind="ExternalOutput")
    E2 = nc.dram_tensor("E2", [B, ROWS], dt.bfloat16, kind="ExternalOutput")
    upd1 = nc.dram_tensor("upd1", [B, D], dt.float32, kind="ExternalOutput")
    upd2 = nc.dram_tensor("upd2", [B, D], dt.float32, kind="ExternalOutput")

    f32 = dt.float32
    f32r = dt.float32r
    Exp = mybir.ActivationFunctionType.Exp

    with tile.TileContext(nc) as tc:
        with (
            tc.tile_pool(name="banks", bufs=1) as banks,
            tc.tile_pool(name="vecs", bufs=1) as vecs,
            tc.tile_pool(name="psum", bufs=6, space="PSUM") as psum,
            tc.tile_pool(name="eout", bufs=4) as eout,
            tc.tile_pool(name="small", bufs=1) as small,
        ):
            t_vT1 = vecs.tile([D, B], f32r, tag="v1")
            t_vT2 = vecs.tile([D, B], f32r, tag="v2")
            nc.sync.dma_start(out=t_vT1[:], in_=vT1.ap())
            nc.sync.dma_start(out=t_vT2[:], in_=vT2.ap())

            # Stream the bank shards in LCHUNK-column pieces.  Each piece's
            # load is issued right before its matmuls so the sync-HWDGE
            # queue interleaves loads with compute instead of front-running
            # all 12.8 MB; E-writebacks go out on the scalar-HWDGE queue so
            # they never queue behind loads.
            LCHUNK = 2500
            NL = ROWS // LCHUNK  # 5
            SUB = LCHUNK // CHUNK  # 5
            for li in range(NL):
                for bank in range(2):
                    src = memT1 if bank == 0 else memT2
                    t_mem = banks.tile([D, LCHUNK], f32r, tag=f"m{bank}_{li}")
                    nc.sync.dma_start(
                        out=t_mem[:],
                        in_=src.ap()[:, li * LCHUNK : (li + 1) * LCHUNK],
                    )
                    t_v = t_vT2 if bank == 0 else t_vT1
                    e_dram = E1 if bank == 0 else E2
                    te = eout.tile([B, LCHUNK], dt.bfloat16, tag="e")
                    for sub in range(SUB):
                        ps = psum.tile([B, CHUNK], f32, tag="ps")
                        nc.tensor.matmul(
                            ps[:],
                            t_v[:],
                            t_mem[:, sub * CHUNK : (sub + 1) * CHUNK],
                            start=True,
                            stop=True,
                        )
                        nc.scalar.activation(
                            te[:, sub * CHUNK : (sub + 1) * CHUNK],
                            ps[:], Exp, scale=1.0 / T,
                        )
                    nc.sync.dma_start(
                        out=e_dram.ap()[:, li * LCHUNK : (li + 1) * LCHUNK],
                        in_=te[:],
                    )

            # momentum update rows (y = arange(B) -> bank rows 0..127):
            # upd = (m + v) / ||m + v||  (the 0.5 momentum factor cancels)
            for bank in range(2):
                t_mh = small.tile([B, D], f32, tag=f"mh{bank}")
                t_vv = small.tile([B, D], f32, tag=f"vv{bank}")
                nc.sync.dma_start(out=t_mh[:], in_=(mh1 if bank == 0 else mh2).ap())
                nc.sync.dma_start(out=t_vv[:], in_=(v1in if bank == 0 else v2in).ap())
                t_sum = small.tile([B, D], f32, tag=f"sum{bank}")
                t_sq = small.tile([B, D], f32, tag=f"sq{bank}")
                t_ss = small.tile([B, 1], f32, tag=f"ss{bank}")
                nc.vector.tensor_tensor(t_sum[:], t_mh[:], t_vv[:], mybir.AluOpType.add)
                nc.vector.tensor_tensor(t_sq[:], t_sum[:], t_sum[:], mybir.AluOpType.mult)
                nc.vector.tensor_reduce(
                    t_ss[:], t_sq[:], mybir.AxisListType.XYZW, mybir.AluOpType.add
                )
                t_norm = small.tile([B, 1], f32, tag=f"n{bank}")
                t_rn = small.tile([B, 1], f32, tag=f"rn{bank}")
                nc.scalar.sqrt(t_norm[:], t_ss[:])
                nc.vector.reciprocal(t_rn[:], t_norm[:])
                t_out = small.tile([B, D], f32, tag=f"o{bank}")
                nc.scalar.activation(
                    t_out[:], t_sum[:], mybir.ActivationFunctionType.Copy,
                    scale=t_rn[:],
                )
                nc.sync.dma_start(
                    out=(upd1 if bank == 0 else upd2).ap(), in_=t_out[:]
                )

    nc.compile()
    return nc


def _get_compiled():
    global _COMPILED
    if _COMPILED is None:
        _COMPILED = _build()
    return _COMPILED


def kernel(v1, v2, idx, y, memory_v1, memory_v2):
    from concourse.bass_utils import run_bass_kernel_spmd

    v1 = np.asarray(v1, dtype=np.float32)
    v2 = np.asarray(v2, dtype=np.float32)
    memory_v1 = np.asarray(memory_v1, dtype=np.float32)
    memory_v2 = np.asarray(memory_v2, dtype=np.float32)
    idx_np = np.asarray(idx)
    y_np = np.asarray(y)

    nc = _get_compiled()

    vT1 = np.ascontiguousarray(v1.T)
    vT2 = np.ascontiguousarray(v2.T)
    mh1 = np.ascontiguousarray(memory_v1[:B])
    mh2 = np.ascontiguousarray(memory_v2[:B])

    in_maps = []
    for c in range(N_CORES):
        r0, r1 = c * ROWS, (c + 1) * ROWS
        in_maps.append({
            "memT1": np.ascontiguousarray(memory_v1[r0:r1].T),
            "memT2": np.ascontiguousarray(memory_v2[r0:r1].T),
            "vT1": vT1, "vT2": vT2,
            "mh1": mh1, "mh2": mh2,
            "v1in": v1, "v2in": v2,
        })

    trace = bool(int(os.environ.get("TRNK_TRACE", "0")))
    if trace:
        try:
            sys.path.insert(0, "/root/.axon_site")
            from trn_agent_boot.trn_boot import _ntff_profile_via_ctypes
            from antenv.axon_hooks import set_axon_ntff_profile_hook
            set_axon_ntff_profile_hook(
                _ntff_profile_via_ctypes("/opt/axon/libaxon_pjrt.so"))
        except Exception as e:
            print(f"trace hook unavailable: {e}")
            trace = False
    res = run_bass_kernel_spmd(nc, in_maps, list(range(N_CORES)), trace=trace)
    if trace:
        kernel.last_exec_ns = res.exec_time_ns

    # ---- host-side unshard/assembly ----
    E1 = np.concatenate(
        [res.results[c]["E1"].astype(np.float32) for c in range(N_CORES)], axis=1)
    E2 = np.concatenate(
        [res.results[c]["E2"].astype(np.float32) for c in range(N_CORES)], axis=1)

    out_v2 = np.take_along_axis(E1, idx_np.astype(np.int64), axis=1)
    out_v1 = np.take_along_axis(E2, idx_np.astype(np.int64), axis=1)
    Z_v1 = np.float32(out_v1.mean()) * np.float32(N)
    Z_v2 = np.float32(out_v2.mean()) * np.float32(N)
    out_v1 = (out_v1 / Z_v1).astype(np.float32)
    out_v2 = (out_v2 / Z_v2).astype(np.float32)

    new_memory_v1 = memory_v1.copy()
    new_memory_v2 = memory_v2.copy()
    # y = arange(B) per the problem spec; honor arbitrary y defensively.
    new_memory_v1[y_np] = res.results[0]["upd1"]
    new_memory_v2[y_np] = res.results[0]["upd2"]

    return out_v1, out_v2, new_memory_v1, new_memory_v2


kernel.last_exec_ns = None
